# revision 7
# baseline (speedup 1.0000x reference)
"""ALIF (adaptive leaky integrate-and-fire) scan on 8 TRN2 NeuronCores.

Problem: tx [T=256, B=128, N=512] f32; per-neuron tau_adp, tau_m [N].
    b   = ro*b + (1-ro)*y
    Bth = 0.3 + 1.8*b
    v   = v*alpha + x - Bth*y
    y   = (v > Bth)
Output: spikes ty [T, B, N] f32.

Strategy: data-parallel over (B x N): 8 cores = 2 b-chunks x 4 n-chunks.
Per core the state is [n=128 partitions, b=64 free] so tau-derived decay
constants are per-partition scalars. The scan is sequential over T; each
step is 3 DVE instructions:
    vp  = ALIF_VP(v, th)  = alpha*v - [v>th]*(rho*th + 2.1*(1-rho))   (custom op)
    v'  = vp + x_t                                                    (tensor_tensor)
    th' = ALIF_TH(th, v)  = rho*th + (1-rho)*([v>th] ? 2.1 : 0.3)     (custom op)
where th = 0.3 + 1.8*b is the tracked threshold state (y = [v > th] is
re-derived inside both ops, so no spike tile is carried). Spike outputs
are materialized in bulk `is_gt` passes per 32-step chunk and DMA'd out.
"""

import numpy as np

T, B, N = 256, 128, 512
PN, FB = 128, 64        # per-core: partitions (n-chunk), free (b-chunk)
NCN, NCB = 4, 2         # n-chunks x b-chunks = 8 cores
CH = 32                 # scan steps per chunk
NCH = T // CH
TH0 = np.float32(0.3) + np.float32(1.8) * np.float32(0.01)  # initial threshold

_CACHE = {}


def _register_custom_ops():
    from concourse.dve_spec import (
        Spec, Src0, Src1, C0, C1, C2, Zero, One, select, lower, _has_src1,
    )
    from concourse.dve_uop import DveOpSpec
    import concourse.dve_ops as dve_ops
    from concourse.dve_ops import DveOp

    def register(name, spec):
        if name in dve_ops._SUB_OPCODE_FOR_NAME:
            return next(op for op in dve_ops.OPS if op.name == name)
        row = dve_ops._CUSTOM_DVE_ROW_BASE + len(dve_ops.OPS)
        assert row < 0x20
        shas = {
            ver: DveOpSpec(
                name=name, opcode=row, uops=lower(spec, ver=ver),
                rd1_en=_has_src1(spec),
            ).sha(ver)
            for ver in ("v3", "v4")
        }
        op = DveOp(name, spec, subdim=False, uops_sha=shas)
        dve_ops.OPS.append(op)
        dve_ops.CUSTOM_DVE_SPECS[name] = spec
        dve_ops._SUB_OPCODE_FOR_NAME[name] = row
        return op

    alif_vp = register(
        "ALIF_VP",
        Spec(
            body=Src0 * C0 - select(Src0 > Src1, Src1 * C1 + (One - C1) * C2, Zero),
            reference=lambda in0, in1, s0, s1, imm2: (
                in0 * s0
                - np.where(in0 > in1, in1 * s1 + (1.0 - s1) * imm2, 0.0)
            ).astype(np.float32),
        ),
    )
    alif_th = register(
        "ALIF_TH",
        Spec(
            body=Src0 * C1 + select(Src1 > Src0, C2, C0) * (One - C1),
            reference=lambda in0, in1, s0, s1, imm2: (
                in0 * s1 + np.where(in1 > in0, imm2, s0) * (1.0 - s1)
            ).astype(np.float32),
        ),
    )
    return alif_vp, alif_th


def _build():
    import concourse.tile as tile
    from concourse import bacc, mybir
    import concourse.mybir as mybir_mod

    alif_vp, alif_th = _register_custom_ops()
    f32 = mybir.dt.float32

    nc = bacc.Bacc("TRN2", target_bir_lowering=False, debug=False)
    x_h = nc.declare_dram_parameter("x", [PN, T, FB], f32, isOutput=False)
    al_h = nc.declare_dram_parameter("alpha", [PN, 1], f32, isOutput=False)
    ro_h = nc.declare_dram_parameter("rho", [PN, 1], f32, isOutput=False)
    o_h = nc.declare_dram_parameter("out", [PN, T, FB], f32, isOutput=True)

    is_gt = mybir_mod.AluOpType.is_gt
    add = mybir_mod.AluOpType.add

    with tile.TileContext(nc) as tc:
        with (
            tc.tile_pool(name="const", bufs=1) as cpool,
            tc.tile_pool(name="xp", bufs=4) as xpool,
            tc.tile_pool(name="vp", bufs=1) as vpool,
            tc.tile_pool(name="tp", bufs=1) as tpool,
            tc.tile_pool(name="sc", bufs=4) as spool,
            tc.tile_pool(name="yp", bufs=2) as ypool,
        ):
            al = cpool.tile([PN, 1], f32, tag="al")
            ro = cpool.tile([PN, 1], f32, tag="ro")
            nc.sync.dma_start(al[:], al_h[:])
            nc.sync.dma_start(ro[:], ro_h[:])

            v0 = cpool.tile([PN, FB], f32, tag="v0")
            th0 = cpool.tile([PN, FB], f32, tag="th0")
            nc.vector.memset(v0[:], 0.0)
            nc.vector.memset(th0[:], float(TH0))

            # prefetch the whole x shard (8 chunks x 1 MiB)
            x_ch = []
            for c in range(NCH):
                xt = xpool.tile([PN, CH * FB], f32, tag="x", name=f"x{c}")
                nc.sync.dma_start(
                    xt[:], x_h[:, c * CH:(c + 1) * CH, :].rearrange("p t f -> p (t f)")
                )
                x_ch.append(xt)

            v_ch = [vpool.tile([PN, CH * FB], f32, tag=f"v{c}", name=f"v{c}") for c in range(NCH)]
            t_ch = [tpool.tile([PN, CH * FB], f32, tag=f"t{c}", name=f"t{c}") for c in range(NCH)]

            def sl(t):
                off = (t % CH) * FB
                return slice(off, off + FB)

            for t in range(T):
                c = t // CH
                if t == 0:
                    v_prev, th_prev = v0[:], th0[:]
                else:
                    pc = (t - 1) // CH
                    v_prev = v_ch[pc][:, sl(t - 1)]
                    th_prev = t_ch[pc][:, sl(t - 1)]
                vp = spool.tile([PN, FB], f32, tag="vp", name=f"vp{t}")
                nc.vector._custom_dve(
                    alif_vp, out=vp[:], in0=v_prev, in1=th_prev,
                    s0=al[:], s1=ro[:], imm2=2.1,
                )
                nc.vector.tensor_tensor(
                    v_ch[c][:, sl(t)], vp[:], x_ch[c][:, sl(t)], add
                )
                nc.vector._custom_dve(
                    alif_th, out=t_ch[c][:, sl(t)], in0=th_prev, in1=v_prev,
                    s0=0.3, s1=ro[:], imm2=2.1,
                )
                if t % CH == CH - 1:
                    y = ypool.tile([PN, CH * FB], f32, tag="y", name=f"y{c}")
                    nc.vector.tensor_tensor(y[:], v_ch[c][:], t_ch[c][:], is_gt)
                    nc.sync.dma_start(
                        o_h[:, c * CH:(c + 1) * CH, :].rearrange("p t f -> p (t f)"),
                        y[:],
                    )

    nc.compile()
    return nc


def _exp_f32(x):
    """f32 exp matching jax-on-cpu as closely as possible."""
    x = np.asarray(x, np.float32)
    try:
        import jax

        cpu = jax.devices("cpu")[0]
        with jax.default_device(cpu):
            import jax.numpy as jnp

            return np.asarray(jax.device_put(jnp.exp(jnp.asarray(x)), cpu))
    except Exception:
        return np.exp(x).astype(np.float32)


def kernel(tx, tau_adp, tau_m):
    from concourse.bass_utils import run_bass_kernel_spmd

    if "nc" not in _CACHE:
        _CACHE["nc"] = _build()
    nc = _CACHE["nc"]

    tx = np.asarray(tx, np.float32)
    alpha = _exp_f32(np.float32(-1.0) / np.asarray(tau_m, np.float32))
    ro = _exp_f32(np.float32(-1.0) / np.asarray(tau_adp, np.float32))

    in_maps = []
    for core in range(8):
        ncn, ncb = core % NCN, core // NCN
        n0, b0 = ncn * PN, ncb * FB
        xs = np.ascontiguousarray(tx[:, b0:b0 + FB, n0:n0 + PN].transpose(2, 0, 1))
        in_maps.append({
            "x": xs,
            "alpha": np.ascontiguousarray(alpha[n0:n0 + PN].reshape(PN, 1)),
            "rho": np.ascontiguousarray(ro[n0:n0 + PN].reshape(PN, 1)),
        })

    res = run_bass_kernel_spmd(nc, in_maps, core_ids=list(range(8)))
    _CACHE["last_result"] = res

    ty = np.empty((T, B, N), np.float32)
    for core in range(8):
        ncn, ncb = core % NCN, core // NCN
        n0, b0 = ncn * PN, ncb * FB
        ty[:, b0:b0 + FB, n0:n0 + PN] = res.results[core]["out"].transpose(1, 2, 0)
    return ty
